# revision 6
# baseline (speedup 1.0000x reference)
"""Trainium2 Bass kernel for LoFTR-style linear attention (nn_AttentionLayer).

Data-parallel over B=1024 across 8 NeuronCores (128 batches/core, no
collectives).  All heavy compute in bf16 on the TensorEngine with fp32 PSUM
accumulation.

Structure (v4):
  - Host-side transpose: pass xg^T (768, 16384) so projections need no
    on-chip transpose (contraction dim = input features on partitions).
  - phi(x) = elu(x)+1 == max(x+1, exp(min(x,0))): ACT Relu + ACT Exp + DVE
    scalar_tensor_tensor max.
  - V augmented with a ones column so KV' = phi(K)^T [V|1] yields both KV and
    Ksum in one matmul; U = phi(Q) @ KV' yields both the unnormalized output
    and the normalizer denominator in one matmul (pairs merged to N=130).
  - Q kept feature-on-partition (bias via ACT per-partition operand);
    K/V token-on-partition (bias folded into the matmul as a K=1 ones-row
    pad matmul - cheaper on PE than a DVE tensor add, since ACT+DVE carry
    all the mandatory PSUM reads and are the secondary bottleneck).
  - Software pipelining: block j+1's Q projection chunks are interleaved
    between block j's batches, and the attention tail (KV'/U) of batch b is
    emitted after the projections of batch b+1, so the PE never waits on the
    ACT/DVE phi chains.
  - The reference's +eps on the denominator is dropped (denominator is a sum
    of strictly positive phi products, O(1e3)).
"""

import numpy as np
import ml_dtypes

NCORES = 8
B, L, HID, GUID, H, D = 1024, 128, 512, 256, 8, 64
KIN = HID + GUID          # 768
BSH = B // NCORES         # 128 batches per core
TOK = BSH * L             # 16384 tokens per core
BLK = 512                 # tokens per block (4 batches)

_CACHE = {}


def _build(nblocks):
    from contextlib import ExitStack
    import concourse.bass as bass
    import concourse.mybir as mybir
    import concourse.tile as tile
    from concourse import bacc

    f32 = mybir.dt.float32
    bf16 = mybir.dt.bfloat16
    AF = mybir.ActivationFunctionType
    OP = mybir.AluOpType

    ntok = nblocks * BLK

    nc = bacc.Bacc("TRN2", target_bir_lowering=False, debug=False,
                   num_devices=NCORES)

    xgT = nc.dram_tensor("xgT", (KIN, ntok), bf16, kind="ExternalInput").ap()
    wq_d = nc.dram_tensor("wq", (128, 6, 512), bf16, kind="ExternalInput").ap()
    wk_d = nc.dram_tensor("wk", (128, 6, 512), bf16, kind="ExternalInput").ap()
    wv_d = nc.dram_tensor("wv", (128, 4, 512), bf16, kind="ExternalInput").ap()
    wk7_d = nc.dram_tensor("wk7", (128, 512), bf16, kind="ExternalInput").ap()
    bvb_d = nc.dram_tensor("bvb", (128, 8, 64), bf16, kind="ExternalInput").ap()
    qb_d = nc.dram_tensor("qb", (128, 4, 2), f32, kind="ExternalInput").ap()
    out_d = nc.dram_tensor("out", (ntok, 512), f32, kind="ExternalOutput").ap()

    with tile.TileContext(nc) as tc, ExitStack() as ctx:
        consts = ctx.enter_context(tc.tile_pool(name="consts", bufs=1))
        xg_pool = ctx.enter_context(tc.tile_pool(name="xg", bufs=3))
        qphi_pool = ctx.enter_context(tc.tile_pool(name="qphi", bufs=2))
        tmp_pool = ctx.enter_context(tc.tile_pool(name="tmp", bufs=8))
        kphi_pool = ctx.enter_context(tc.tile_pool(name="kphi", bufs=2))
        out_pool = ctx.enter_context(tc.tile_pool(name="outp", bufs=4))
        rcp_pool = ctx.enter_context(tc.tile_pool(name="rcp", bufs=4))
        psq_pool = ctx.enter_context(tc.tile_pool(name="psq", bufs=2, space="PSUM"))
        pskv_pool = ctx.enter_context(tc.tile_pool(name="pskv", bufs=3, space="PSUM"))
        psatt_pool = ctx.enter_context(tc.tile_pool(name="psatt", bufs=1, space="PSUM"))
        psu_pool = ctx.enter_context(tc.tile_pool(name="psu", bufs=1, space="PSUM"))

        wq_t = consts.tile([128, 6, 512], bf16)
        wk_t = consts.tile([128, 6, 512], bf16)
        wv_t = consts.tile([128, 4, 512], bf16)
        wk7_t = consts.tile([128, 512], bf16)
        bvb_t = consts.tile([128, 8, 64], bf16)
        qb_t = consts.tile([128, 4, 2], f32)
        pad_t = consts.tile([128, 128], bf16)
        for k in range(6):
            nc.sync.dma_start(wq_t[:, k, :], wq_d[:, k, :])
            nc.sync.dma_start(wk_t[:, k, :], wk_d[:, k, :])
            if k < 4:
                nc.sync.dma_start(wv_t[:, k, :], wv_d[:, k, :])
        nc.sync.dma_start(wk7_t[:], wk7_d[:])
        nc.sync.dma_start(bvb_t[:], bvb_d[:])
        nc.sync.dma_start(qb_t[:], qb_d[:])
        # pad_t.T @ wk7 adds the bk bias row to every token: row 0 is ones,
        # rows 1..127 zero; wk7 row 0 holds bk.
        nc.vector.memset(pad_t[:], 0.0)
        nc.vector.memset(pad_t[0:1, :], 1.0)

        # KV' staging: E holds even heads on partitions 0:64 (cols 0:65), O
        # holds odd heads on partitions 64:128 (cols 65:130); the other halves
        # are statically zero (matmuls with operands at SBUF base partition 64
        # crash, so U matmuls run full K=128 against zero-padded rhs).
        kvEO = [consts.tile([128, 4, 130], bf16, name=f"kvEO{i}")
                for i in range(2)]
        for i in range(2):
            nc.vector.memset(kvEO[i][64:128, :, 0:65], 0.0)
            nc.vector.memset(kvEO[i][0:64, :, 65:130], 0.0)
        # V' double buffers with the ones column (for Ksum) set statically.
        vpb = [consts.tile([128, 8, 65], bf16, name=f"vp{i}") for i in range(2)]
        for i in range(2):
            nc.vector.memset(vpb[i][:, :, 64:65], 1.0)

        def emit_qchunk(xg_t, qphi_t, m):
            # Q projection chunk m (feature-on-partition) + phi
            ps = psq_pool.tile([128, 512], f32, tag="psq")
            for k in range(6):
                nc.tensor.matmul(
                    ps[:],
                    wq_t[:, k, m * 128:(m + 1) * 128],
                    xg_t[:, k, :],
                    start=(k == 0), stop=(k == 5),
                )
            u = tmp_pool.tile([128, 512], bf16, tag="tmp")
            nc.scalar.activation(u[:], ps[:], AF.Relu,
                                 bias=qb_t[:, m, 0:1], scale=-1.0)
            e = tmp_pool.tile([128, 512], bf16, tag="tmp")
            nc.scalar.activation(e[:], u[:], AF.Exp, scale=-1.0)
            nc.vector.scalar_tensor_tensor(
                qphi_t[:, m, :], ps[:], qb_t[:, m, 1:2], e[:],
                OP.add, OP.max,
            )

        def emit_proj(xg_t, bi):
            # K/V projections for one batch + phi(K) chain + V' assembly.
            bs = slice(bi * 128, (bi + 1) * 128)
            ps_k = pskv_pool.tile([128, 512], f32, tag="pskv")
            for k in range(6):
                nc.tensor.matmul(ps_k[:], xg_t[:, k, bs], wk_t[:, k, :],
                                 start=(k == 0), stop=False)
            nc.tensor.matmul(ps_k[:], pad_t[:], wk7_t[:],
                             start=False, stop=True)

            ps_v = pskv_pool.tile([128, 512], f32, tag="pskv")
            for k in range(4):
                nc.tensor.matmul(ps_v[:], xg_t[:, k, bs], wv_t[:, k, :],
                                 start=(k == 0), stop=(k == 3))

            u = tmp_pool.tile([128, 512], bf16, tag="tmp")
            nc.scalar.activation(u[:], ps_k[:], AF.Relu, scale=-1.0)
            e = tmp_pool.tile([128, 512], bf16, tag="tmp")
            nc.scalar.activation(e[:], u[:], AF.Exp, scale=-1.0)
            kphi_t = kphi_pool.tile([128, 512], bf16, tag="kphi")
            nc.vector.scalar_tensor_tensor(
                kphi_t[:], ps_k[:], 1.0, e[:], OP.add, OP.max)

            vp_t = vpb[bi % 2]
            nc.vector.tensor_tensor(
                vp_t[:, :, 0:64],
                ps_v[:].rearrange("p (h d) -> p h d", d=64),
                bvb_t[:], OP.add)
            return kphi_t, vp_t

        def emit_attn(qphi_t, kphi_t, vp_t, b, bi):
            # KV' = phi(K)^T @ V' then U = phi(Q) @ KV' and normalization.
            bs = slice(bi * 128, (bi + 1) * 128)
            ps_kv_full = psatt_pool.tile([128, 512], f32, tag="psatt",
                                         name="ps_kv")
            ps_kv = ps_kv_full[:, :260]
            for p in range(4):
                nc.tensor.matmul(
                    ps_kv[0:64, p * 65:(p + 1) * 65],
                    kphi_t[:, p * 128:p * 128 + 64],
                    vp_t[:, 2 * p, :],
                    start=True, stop=True, tile_position=(0, 0))
                nc.tensor.matmul(
                    ps_kv[64:128, p * 65:(p + 1) * 65],
                    kphi_t[:, p * 128 + 64:(p + 1) * 128],
                    vp_t[:, 2 * p + 1, :],
                    start=True, stop=True, tile_position=(0, 64))
            kv_t = kvEO[b % 2]
            nc.scalar.copy(
                kv_t[0:64, :, 0:65],
                ps_kv[0:64, :].rearrange("p (c j) -> p c j", j=65))
            nc.vector.tensor_copy(
                kv_t[64:128, :, 65:130],
                ps_kv[64:128, :].rearrange("p (c j) -> p c j", j=65))

            # U for all 8 heads into one 2-bank PSUM tile: halves at column
            # offsets 0 and 512, each 2 head-pairs x 130.
            ps_u_full = psu_pool.tile([128, 1024], f32, tag="psu", name="ps_u")
            for half in range(2):
                for pp in range(2):
                    p = half * 2 + pp
                    nc.tensor.matmul(
                        ps_u_full[:, half * 512 + pp * 130:
                                  half * 512 + (pp + 1) * 130],
                        qphi_t[:, p, bs],
                        kv_t[:, p, :],
                        start=True, stop=True)
            ps_u4 = ps_u_full[:].rearrange(
                "p (g x) -> p g x", g=2)[:, :, 0:260].rearrange(
                "p g (c j) -> p g c j", j=65)
            # denominator is a sum of strictly-positive phi products (O(1e3)),
            # so the reference's +eps is numerically irrelevant
            r_t = rcp_pool.tile([128, 2, 4], f32, tag="rcp")
            nc.vector.reciprocal(r_t[:], ps_u4[:, :, :, 64])
            out_t = out_pool.tile([128, 512], f32, tag="outp")
            nc.vector.tensor_tensor(
                out_t[:].rearrange("p (g c d) -> p g c d", g=2, d=64),
                ps_u4[:, :, :, 0:64],
                r_t[:, :, :, None].to_broadcast((128, 2, 4, 64)),
                OP.mult,
            )
            nc.sync.dma_start(out_d[b * 128:(b + 1) * 128, :], out_t[:])

        def load_xg(j):
            xg_t = xg_pool.tile([128, 6, 512], bf16, tag="xg")
            xgv = xgT[:, j * BLK:(j + 1) * BLK].rearrange(
                "(ko p) n -> p ko n", p=128)
            for k in range(6):
                nc.sync.dma_start(xg_t[:, k, :], xgv[:, k, :])
            return xg_t

        # prologue: block 0's inputs + Q projection
        xg_blk = {0: load_xg(0)}
        qphi_blk = {0: qphi_pool.tile([128, 4, 512], bf16, tag="qphi",
                                      name="qphi_t")}
        for m in range(4):
            emit_qchunk(xg_blk[0], qphi_blk[0], m)

        pending = None  # (qphi_t, kphi_t, vp_t, b, bi) awaiting attn tail
        for j in range(nblocks):
            if j + 1 < nblocks:
                xg_blk[j + 1] = load_xg(j + 1)
                qphi_blk[j + 1] = qphi_pool.tile([128, 4, 512], bf16,
                                                 tag="qphi", name="qphi_t")
            for bi in range(4):
                b = j * 4 + bi
                kphi_t, vp_t = emit_proj(xg_blk[j], bi)
                if pending is not None:
                    emit_attn(*pending)
                pending = (qphi_blk[j], kphi_t, vp_t, b, bi)
                if j + 1 < nblocks:
                    emit_qchunk(xg_blk[j + 1], qphi_blk[j + 1], bi)
            xg_blk.pop(j - 1, None)
            qphi_blk.pop(j - 1, None)
        emit_attn(*pending)

    nc.compile()
    return nc


def _get_nc(nblocks=TOK // BLK):
    if nblocks not in _CACHE:
        _CACHE[nblocks] = _build(nblocks)
    return _CACHE[nblocks]


def _prep_shared(Wq, bq, Wk, bk, Wv, bv):
    bf = ml_dtypes.bfloat16
    wq = np.ascontiguousarray(
        Wq.reshape(6, 128, 512).transpose(1, 0, 2)).astype(bf)
    wk = np.ascontiguousarray(
        Wk.reshape(6, 128, 512).transpose(1, 0, 2)).astype(bf)
    wv = np.ascontiguousarray(
        Wv.reshape(4, 128, 512).transpose(1, 0, 2)).astype(bf)
    wk7 = np.zeros((128, 512), np.float32)
    wk7[0, :] = bk
    wk7 = wk7.astype(bf)
    bvb = np.ascontiguousarray(
        np.broadcast_to(bv.reshape(8, 64), (128, 8, 64))).astype(bf)
    qb = np.ascontiguousarray(np.stack(
        [(-bq).reshape(4, 128).T, (bq + 1.0).reshape(4, 128).T],
        axis=-1)).astype(np.float32)
    return {"wq": wq, "wk": wk, "wv": wv, "wk7": wk7, "bvb": bvb, "qb": qb}


def kernel(x, guidance, Wq, bq, Wk, bk, Wv, bv):
    from concourse.bass_utils import run_bass_kernel_spmd

    x = np.asarray(x, dtype=np.float32)
    guidance = np.asarray(guidance, dtype=np.float32)
    Wq = np.asarray(Wq, dtype=np.float32)
    bq = np.asarray(bq, dtype=np.float32)
    Wk = np.asarray(Wk, dtype=np.float32)
    bk = np.asarray(bk, dtype=np.float32)
    Wv = np.asarray(Wv, dtype=np.float32)
    bv = np.asarray(bv, dtype=np.float32)

    nc = _get_nc()
    shared = _prep_shared(Wq, bq, Wk, bk, Wv, bv)
    bf = ml_dtypes.bfloat16

    in_maps = []
    for c in range(NCORES):
        xs = np.asarray(x[c * BSH:(c + 1) * BSH]).reshape(TOK, HID)
        gs = np.asarray(guidance[c * BSH:(c + 1) * BSH]).reshape(TOK, GUID)
        xg = np.concatenate([xs, gs], axis=1)
        xgT = np.ascontiguousarray(xg.T).astype(bf)
        in_maps.append({"xgT": xgT, **shared})

    res = run_bass_kernel_spmd(nc, in_maps, core_ids=list(range(NCORES)))
    outs = [r["out"] for r in res.results]
    return np.concatenate(outs, axis=0).reshape(B, L, H * D).astype(np.float32)


# revision 7
# speedup vs baseline: 1.0629x; 1.0629x over previous
"""Trainium2 Bass kernel for LoFTR-style linear attention (nn_AttentionLayer).

Data-parallel over B=1024 across 8 NeuronCores (128 batches/core, no
collectives).  All heavy compute in bf16 on the TensorEngine with fp32 PSUM
accumulation.

Structure (v5):
  - Host-side retile: xg^T is passed as (nblocks, 6, 128, 512) and the
    weights as (kslices, 128, 512), so every DMA transfer is fully
    contiguous in HBM (the strided row loads of v4 cost ~9us per slice and
    put a 21us DMA head on the kernel).
  - phi(x) = elu(x)+1 == max(x+1, exp(min(x,0))): ACT Relu + ACT Exp + DVE
    scalar_tensor_tensor max.
  - V augmented with a ones column so KV' = phi(K)^T [V|1] yields both KV and
    Ksum in one matmul; U = phi(Q) @ KV' yields both the unnormalized output
    and the normalizer denominator in one matmul (pairs merged to N=130).
  - Q kept feature-on-partition (bias via ACT per-partition operand);
    K/V token-on-partition.  The K bias is a DVE tensor add (bkb broadcast
    tile) instead of v4's ones-row pad matmul: PE is the roofline engine, so
    27us of pad matmuls moves to the (slacker) DVE.
  - Software pipelining: block j+1's Q projection chunks are interleaved
    between block j's batches, and the attention tail (KV'/U) of batch b is
    emitted after the projections of batch b+1, so the PE never waits on the
    ACT/DVE phi chains.
  - The reference's +eps on the denominator is dropped (denominator is a sum
    of strictly positive phi products, O(1e3)).
"""

import numpy as np
import ml_dtypes

NCORES = 8
B, L, HID, GUID, H, D = 1024, 128, 512, 256, 8, 64
KIN = HID + GUID          # 768
BSH = B // NCORES         # 128 batches per core
TOK = BSH * L             # 16384 tokens per core
BLK = 512                 # tokens per block (4 batches)
NBLK = TOK // BLK

_CACHE = {}


def _build(nblocks):
    from contextlib import ExitStack
    import concourse.bass as bass
    import concourse.mybir as mybir
    import concourse.tile as tile
    from concourse import bacc

    f32 = mybir.dt.float32
    bf16 = mybir.dt.bfloat16
    AF = mybir.ActivationFunctionType
    OP = mybir.AluOpType

    ntok = nblocks * BLK

    nc = bacc.Bacc("TRN2", target_bir_lowering=False, debug=False,
                   num_devices=NCORES)

    xg_d = nc.dram_tensor("xgb", (nblocks, 6, 128, 512), bf16,
                          kind="ExternalInput").ap()
    wq_d = nc.dram_tensor("wq", (6, 128, 512), bf16, kind="ExternalInput").ap()
    wk_d = nc.dram_tensor("wk", (6, 128, 512), bf16, kind="ExternalInput").ap()
    wv_d = nc.dram_tensor("wv", (4, 128, 512), bf16, kind="ExternalInput").ap()
    bkb_d = nc.dram_tensor("bkb", (128, 512), bf16, kind="ExternalInput").ap()
    bvb_d = nc.dram_tensor("bvb", (128, 8, 64), bf16, kind="ExternalInput").ap()
    qb_d = nc.dram_tensor("qb", (128, 4, 2), f32, kind="ExternalInput").ap()
    out_d = nc.dram_tensor("out", (ntok, 512), f32, kind="ExternalOutput").ap()

    with tile.TileContext(nc) as tc, ExitStack() as ctx:
        consts = ctx.enter_context(tc.tile_pool(name="consts", bufs=1))
        xg_pool = ctx.enter_context(tc.tile_pool(name="xg", bufs=3))
        qphi_pool = ctx.enter_context(tc.tile_pool(name="qphi", bufs=2))
        tmp_pool = ctx.enter_context(tc.tile_pool(name="tmp", bufs=8))
        kphi_pool = ctx.enter_context(tc.tile_pool(name="kphi", bufs=2))
        out_pool = ctx.enter_context(tc.tile_pool(name="outp", bufs=4))
        rcp_pool = ctx.enter_context(tc.tile_pool(name="rcp", bufs=4))
        psq_pool = ctx.enter_context(tc.tile_pool(name="psq", bufs=2, space="PSUM"))
        pskv_pool = ctx.enter_context(tc.tile_pool(name="pskv", bufs=3, space="PSUM"))
        psatt_pool = ctx.enter_context(tc.tile_pool(name="psatt", bufs=1, space="PSUM"))
        psu_pool = ctx.enter_context(tc.tile_pool(name="psu", bufs=1, space="PSUM"))

        wq_t = consts.tile([128, 6, 512], bf16)
        wk_t = consts.tile([128, 6, 512], bf16)
        wv_t = consts.tile([128, 4, 512], bf16)
        bkb_t = consts.tile([128, 512], bf16)
        bvb_t = consts.tile([128, 8, 64], bf16)
        qb_t = consts.tile([128, 4, 2], f32)

        def load_xg(j):
            xg_t = xg_pool.tile([128, 6, 512], bf16, tag="xg", name="xg_t")
            for k in range(6):
                nc.sync.dma_start(xg_t[:, k, :], xg_d[j, k, :, :])
            return xg_t

        # Issue order puts the transfers the first Q chunk waits on (xg(0)
        # slice k, wq slice k) on the earliest DMA queues, alternating.
        xg_blk = {}
        xg_t0 = xg_pool.tile([128, 6, 512], bf16, tag="xg", name="xg_t")
        for k in range(6):
            nc.sync.dma_start(xg_t0[:, k, :], xg_d[0, k, :, :])
            nc.sync.dma_start(wq_t[:, k, :], wq_d[k, :, :])
        xg_blk[0] = xg_t0
        for k in range(6):
            nc.sync.dma_start(wk_t[:, k, :], wk_d[k, :, :])
            if k < 4:
                nc.sync.dma_start(wv_t[:, k, :], wv_d[k, :, :])
        nc.sync.dma_start(bkb_t[:], bkb_d[:])
        nc.sync.dma_start(bvb_t[:], bvb_d[:])
        nc.sync.dma_start(qb_t[:], qb_d[:])

        # KV' staging: E holds even heads on partitions 0:64 (cols 0:65), O
        # holds odd heads on partitions 64:128 (cols 65:130); the other halves
        # are statically zero (matmuls with operands at SBUF base partition 64
        # crash, so U matmuls run full K=128 against zero-padded rhs).
        kvEO = [consts.tile([128, 4, 130], bf16, name=f"kvEO{i}")
                for i in range(2)]
        for i in range(2):
            nc.vector.memset(kvEO[i][64:128, :, 0:65], 0.0)
            nc.vector.memset(kvEO[i][0:64, :, 65:130], 0.0)
        # V' double buffers with the ones column (for Ksum) set statically.
        vpb = [consts.tile([128, 8, 65], bf16, name=f"vp{i}") for i in range(2)]
        for i in range(2):
            nc.vector.memset(vpb[i][:, :, 64:65], 1.0)

        def emit_qchunk(xg_t, qphi_t, m):
            # Q projection chunk m (feature-on-partition) + phi
            ps = psq_pool.tile([128, 512], f32, tag="psq")
            for k in range(6):
                nc.tensor.matmul(
                    ps[:],
                    wq_t[:, k, m * 128:(m + 1) * 128],
                    xg_t[:, k, :],
                    start=(k == 0), stop=(k == 5),
                )
            u = tmp_pool.tile([128, 512], bf16, tag="tmp")
            nc.scalar.activation(u[:], ps[:], AF.Relu,
                                 bias=qb_t[:, m, 0:1], scale=-1.0)
            e = tmp_pool.tile([128, 512], bf16, tag="tmp")
            nc.scalar.activation(e[:], u[:], AF.Exp, scale=-1.0)
            nc.vector.scalar_tensor_tensor(
                qphi_t[:, m, :], ps[:], qb_t[:, m, 1:2], e[:],
                OP.add, OP.max,
            )

        def emit_proj(xg_t, bi):
            # K/V projections for one batch + phi(K) chain + V' assembly.
            bs = slice(bi * 128, (bi + 1) * 128)
            ps_k = pskv_pool.tile([128, 512], f32, tag="pskv")
            for k in range(6):
                nc.tensor.matmul(ps_k[:], xg_t[:, k, bs], wk_t[:, k, :],
                                 start=(k == 0), stop=(k == 5))
            z = tmp_pool.tile([128, 512], bf16, tag="tmp")
            nc.vector.tensor_tensor(z[:], ps_k[:], bkb_t[:], OP.add)

            ps_v = pskv_pool.tile([128, 512], f32, tag="pskv")
            for k in range(4):
                nc.tensor.matmul(ps_v[:], xg_t[:, k, bs], wv_t[:, k, :],
                                 start=(k == 0), stop=(k == 3))

            u = tmp_pool.tile([128, 512], bf16, tag="tmp")
            nc.scalar.activation(u[:], z[:], AF.Relu, scale=-1.0)
            e = tmp_pool.tile([128, 512], bf16, tag="tmp")
            nc.scalar.activation(e[:], u[:], AF.Exp, scale=-1.0)
            kphi_t = kphi_pool.tile([128, 512], bf16, tag="kphi")
            nc.vector.scalar_tensor_tensor(
                kphi_t[:], z[:], 1.0, e[:], OP.add, OP.max)

            vp_t = vpb[bi % 2]
            nc.vector.tensor_tensor(
                vp_t[:, :, 0:64],
                ps_v[:].rearrange("p (h d) -> p h d", d=64),
                bvb_t[:], OP.add)
            return kphi_t, vp_t

        def emit_attn(qphi_t, kphi_t, vp_t, b, bi):
            # KV' = phi(K)^T @ V' then U = phi(Q) @ KV' and normalization.
            bs = slice(bi * 128, (bi + 1) * 128)
            ps_kv_full = psatt_pool.tile([128, 512], f32, tag="psatt",
                                         name="ps_kv")
            ps_kv = ps_kv_full[:, :260]
            for p in range(4):
                nc.tensor.matmul(
                    ps_kv[0:64, p * 65:(p + 1) * 65],
                    kphi_t[:, p * 128:p * 128 + 64],
                    vp_t[:, 2 * p, :],
                    start=True, stop=True, tile_position=(0, 0))
                nc.tensor.matmul(
                    ps_kv[64:128, p * 65:(p + 1) * 65],
                    kphi_t[:, p * 128 + 64:(p + 1) * 128],
                    vp_t[:, 2 * p + 1, :],
                    start=True, stop=True, tile_position=(0, 64))
            kv_t = kvEO[b % 2]
            nc.scalar.copy(
                kv_t[0:64, :, 0:65],
                ps_kv[0:64, :].rearrange("p (c j) -> p c j", j=65))
            nc.vector.tensor_copy(
                kv_t[64:128, :, 65:130],
                ps_kv[64:128, :].rearrange("p (c j) -> p c j", j=65))

            # U for all 8 heads into one 2-bank PSUM tile: halves at column
            # offsets 0 and 512, each 2 head-pairs x 130.
            ps_u_full = psu_pool.tile([128, 1024], f32, tag="psu", name="ps_u")
            for half in range(2):
                for pp in range(2):
                    p = half * 2 + pp
                    nc.tensor.matmul(
                        ps_u_full[:, half * 512 + pp * 130:
                                  half * 512 + (pp + 1) * 130],
                        qphi_t[:, p, bs],
                        kv_t[:, p, :],
                        start=True, stop=True)
            ps_u4 = ps_u_full[:].rearrange(
                "p (g x) -> p g x", g=2)[:, :, 0:260].rearrange(
                "p g (c j) -> p g c j", j=65)
            # denominator is a sum of strictly-positive phi products (O(1e3)),
            # so the reference's +eps is numerically irrelevant
            r_t = rcp_pool.tile([128, 2, 4], f32, tag="rcp")
            nc.vector.reciprocal(r_t[:], ps_u4[:, :, :, 64])
            out_t = out_pool.tile([128, 512], f32, tag="outp")
            nc.vector.tensor_tensor(
                out_t[:].rearrange("p (g c d) -> p g c d", g=2, d=64),
                ps_u4[:, :, :, 0:64],
                r_t[:, :, :, None].to_broadcast((128, 2, 4, 64)),
                OP.mult,
            )
            nc.sync.dma_start(out_d[b * 128:(b + 1) * 128, :], out_t[:])

        # prologue: block 0's Q projection
        qphi_blk = {0: qphi_pool.tile([128, 4, 512], bf16, tag="qphi",
                                      name="qphi_t")}
        for m in range(4):
            emit_qchunk(xg_blk[0], qphi_blk[0], m)

        pending = None  # (qphi_t, kphi_t, vp_t, b, bi) awaiting attn tail
        for j in range(nblocks):
            if j + 1 < nblocks:
                xg_blk[j + 1] = load_xg(j + 1)
                qphi_blk[j + 1] = qphi_pool.tile([128, 4, 512], bf16,
                                                 tag="qphi", name="qphi_t")
            for bi in range(4):
                b = j * 4 + bi
                kphi_t, vp_t = emit_proj(xg_blk[j], bi)
                if pending is not None:
                    emit_attn(*pending)
                pending = (qphi_blk[j], kphi_t, vp_t, b, bi)
                if j + 1 < nblocks:
                    emit_qchunk(xg_blk[j + 1], qphi_blk[j + 1], bi)
            xg_blk.pop(j - 1, None)
            qphi_blk.pop(j - 1, None)
        emit_attn(*pending)

    nc.compile()
    return nc


def _get_nc(nblocks=NBLK):
    if nblocks not in _CACHE:
        _CACHE[nblocks] = _build(nblocks)
    return _CACHE[nblocks]


def _prep_shared(Wq, bq, Wk, bk, Wv, bv):
    bf = ml_dtypes.bfloat16
    wq = np.ascontiguousarray(Wq.reshape(6, 128, 512)).astype(bf)
    wk = np.ascontiguousarray(Wk.reshape(6, 128, 512)).astype(bf)
    wv = np.ascontiguousarray(Wv.reshape(4, 128, 512)).astype(bf)
    bkb = np.ascontiguousarray(
        np.broadcast_to(bk.reshape(1, 512), (128, 512))).astype(bf)
    bvb = np.ascontiguousarray(
        np.broadcast_to(bv.reshape(8, 64), (128, 8, 64))).astype(bf)
    qb = np.ascontiguousarray(np.stack(
        [(-bq).reshape(4, 128).T, (bq + 1.0).reshape(4, 128).T],
        axis=-1)).astype(np.float32)
    return {"wq": wq, "wk": wk, "wv": wv, "bkb": bkb, "bvb": bvb, "qb": qb}


def _prep_xg(x_c, g_c):
    # (BSH, L, HID) + (BSH, L, GUID) -> (NBLK, 6, 128, 512) bf16 block-tiled
    bf = ml_dtypes.bfloat16
    xg = np.concatenate([x_c.reshape(TOK, HID), g_c.reshape(TOK, GUID)],
                        axis=1)                      # (TOK, 768)
    xgT = xg.T.reshape(6, 128, NBLK, 512)            # (k, p, j, t)
    return np.ascontiguousarray(xgT.transpose(2, 0, 1, 3)).astype(bf)


def kernel(x, guidance, Wq, bq, Wk, bk, Wv, bv):
    from concourse.bass_utils import run_bass_kernel_spmd

    x = np.asarray(x, dtype=np.float32)
    guidance = np.asarray(guidance, dtype=np.float32)
    Wq = np.asarray(Wq, dtype=np.float32)
    bq = np.asarray(bq, dtype=np.float32)
    Wk = np.asarray(Wk, dtype=np.float32)
    bk = np.asarray(bk, dtype=np.float32)
    Wv = np.asarray(Wv, dtype=np.float32)
    bv = np.asarray(bv, dtype=np.float32)

    nc = _get_nc()
    shared = _prep_shared(Wq, bq, Wk, bk, Wv, bv)

    in_maps = []
    for c in range(NCORES):
        xgb = _prep_xg(x[c * BSH:(c + 1) * BSH],
                       guidance[c * BSH:(c + 1) * BSH])
        in_maps.append({"xgb": xgb, **shared})

    res = run_bass_kernel_spmd(nc, in_maps, core_ids=list(range(NCORES)))
    outs = [r["out"] for r in res.results]
    return np.concatenate(outs, axis=0).reshape(B, L, H * D).astype(np.float32)
